# revision 27
# baseline (speedup 1.0000x reference)
"""Trainium2 Bass kernel for nn_CIN (Compressed Interaction Network).

Math (per layer k, x0 = x fixed):
    x_{k+1}[b,h,d] = sum_{i,j} W[i,j,h] * x0[b,i,d] * xk[b,j,d]
    outs_k[b,h]    = sum_d x_{k+1}[b,h,d]
    output = concat(outs_0, outs_1, outs_2)   # [B, 384]

Strategy (pure data parallel over batch, 8 cores x 128 batches):
  - bf16 compute, fp32 PSUM accumulation (fp8 was simulated at rel err
    0.033-0.067 -- fails the 2e-2 gate, so bf16 everywhere).
  - Per core, 8 blocks of 16 batches; free dim F = 16*64 = 1024 (b,d).
  - Layer 0 uses the i<=j symmetry: 820 unique pairs, W0sym = W0[i,j]+W0[j,i],
    and the pair PRODUCTS are built on the host (p0suf = x[i]*x[j] gather
    image, bf16, partition-major): halves the suf DMA bytes vs shipping two
    operand images and removes the L0 DVE products entirely.
  - Layer 1 products P[(i,j), f] = x0[i,f]*x1[j,f] on DVE (2x bf16 mode,
    ~2.13us per 4-chunk group). x0 rows broadcast across partitions (REP):
    7 groups via DMA stride-0 partition-broadcast APs alternating the two
    HWDGE rings, 2 groups (6,7) via PE ones-matmul + ACT PSUM copy, and the
    last group (9) via the pool engine's partition_broadcast -- its ~8us
    (up to ~21us jitter) runtime hides behind a ~24us deadline, relieving
    both the DMA rings and the PE. outs_0 reduce also rides on the pool.
  - Matmuls: stationary = W chunk [c,h], moving = P chunk [c, 512] (PSUM
    write cap), emitted 512-half-first in layer 0 so the x1 copy / first
    L1 product chain overlaps the second half's matmuls. PSUM accumulation
    -> x_{k+1} in [h, (b,d)] layout = next layer's input layout.
  - Layer 2 never materializes x3: outs_2 = W2 : G2 where
    G2'[b][j,i] = sum_d x2[b,j,d]*x0[b,i,d] (per-batch Gram via PE), then
    one 40-chunk contraction. Batch PAIRS share one 128-row transpose and
    one gram matmul against a host-built block-diagonal xdtP (even batch in
    partitions 0-63 / cols 0-40, odd in 64-127 / 41-81), halving the L2 PE
    instruction count; a ones-column per half makes the gram emit outs_1
    for free.
  - Engine budget per block (~modeled): PE ~27us (14 L0 + 80 L1 + 16 REP
    matmuls + 8 transposes + 8 grams), DMA ~25.5us (1.84MB p0 + 7x1.05MB
    rep), DVE ~21.3us (10 product groups), pool ~10us, ACT ~14us. A
    32-matmul warm-up spin covers the DMA-only startup window (PE clock
    ramps 1.2->2.4GHz over ~3.4us of sustained activity).
"""
import os
import sys

sys.path.insert(0, "/opt/trn_rl_repo")
os.environ.setdefault("JAX_PLATFORMS", "cpu")

from contextlib import ExitStack

import numpy as np
import ml_dtypes

import concourse.bass as bass  # noqa: F401
import concourse.tile as tile
from concourse import bacc, library_config, mybir
from concourse.bass_utils import run_bass_kernel_spmd

BF16 = mybir.dt.bfloat16
F32 = mybir.dt.float32
NPBF16 = ml_dtypes.bfloat16

B, M, D, HK = 1024, 40, 64, 128
NCORE = 8
BS = B // NCORE          # 128 batches per core
NBLK = 8                 # blocks per core
BB = BS // NBLK          # 16 batches per block
F = BB * D               # 1024 free elements per block
NI = M                   # 40 chunks in layer 1
GRP = 4                  # chunks fused per group (one TT / one REP DMA)
NGRP = NI // GRP         # 10 groups in layer 1
NP0 = M * (M + 1) // 2   # 820 unique layer-0 pairs
NCH0 = 7                 # layer-0 chunks (820 -> 896 rows, 76 zero-pad)
GRP0S = (4, 3)           # layer-0 group sizes (4 + 3 chunks)
NMM = F // 512           # matmuls per chunk (PSUM write = 512 els max)
HB = BB // 2             # L2 gram half-size (8 batches -> 4 pair-grams)
NPR = HB // 2            # batch pairs per L2 half-call
MC = M + 1               # gram columns per batch (40 i's + ones)
PE_REP = (7,)            # REP groups built by PE ones-matmul
POOL_BC = (9,)           # REP groups built by pool partition_broadcast

_PROFILE = False
_TRACE_KW = {}
_nc_cache = None
_last_results = None


def _build():
    nc = bacc.Bacc("TRN2", target_bir_lowering=False, debug=False,
                   enable_asserts=False)

    p0suf_d = nc.dram_tensor("p0suf", [NBLK, 128, NCH0, F], BF16,
                             kind="ExternalInput").ap()
    x0f_d = nc.dram_tensor("x0f", [NBLK, NI, F], BF16, kind="ExternalInput").ap()
    xdtp_d = nc.dram_tensor("xdtp", [128, BS // 2, 2 * MC], BF16,
                            kind="ExternalInput").ap()
    w0_d = nc.dram_tensor("w0", [128, NCH0, HK], BF16, kind="ExternalInput").ap()
    w1_d = nc.dram_tensor("w1", [128, NI, HK], BF16, kind="ExternalInput").ap()
    w2_d = nc.dram_tensor("w2", [128, NI, HK], BF16, kind="ExternalInput").ap()
    idb_d = nc.dram_tensor("idb", [128, 128], BF16, kind="ExternalInput").ap()
    idf_d = nc.dram_tensor("idf", [128, 128], F32, kind="ExternalInput").ap()
    out_d = nc.dram_tensor("out", [BS, 3 * HK], F32, kind="ExternalOutput").ap()

    with tile.TileContext(nc) as tc, ExitStack() as ctx:
        stat = ctx.enter_context(tc.tile_pool(name="stat", bufs=1))
        sufp = ctx.enter_context(tc.tile_pool(name="sufp", bufs=4))
        # one buffer per non-pool rep group: the PE-REP ACT copies and the
        # trailing DMA groups must never wait on a buffer still being
        # consumed by this block's early products (that stall resets the PE
        # clock ramp); pool-bcast rep tiles get their own pool so the
        # broadcast can start at block top.
        repp = ctx.enter_context(tc.tile_pool(name="repp", bufs=8))
        repbc = ctx.enter_context(tc.tile_pool(name="repbc", bufs=2))
        x0pp = ctx.enter_context(tc.tile_pool(name="x0pp", bufs=2))
        pp = ctx.enter_context(tc.tile_pool(name="pp", bufs=2))
        xkp = ctx.enter_context(tc.tile_pool(name="xkp", bufs=4))
        x2tp = ctx.enter_context(tc.tile_pool(name="x2tp", bufs=2))
        ps_acc = ctx.enter_context(tc.tile_pool(name="ps_acc", bufs=2, space="PSUM"))
        ps_tr = ctx.enter_context(tc.tile_pool(name="ps_tr", bufs=1, space="PSUM"))
        ps_sm = ctx.enter_context(tc.tile_pool(name="ps_sm", bufs=1, space="PSUM"))
        ps_rp = ctx.enter_context(tc.tile_pool(name="ps_rp", bufs=2, space="PSUM"))

        # critical loads first (w0/w1 are fat contiguous descriptors);
        # remaining static tensors stream in behind block-0/1 traffic
        w0sb = stat.tile([128, NCH0, HK], BF16, tag="w0sb")
        nc.sync.dma_start(w0sb[:], w0_d[:])
        w1sb = stat.tile([128, NI, HK], BF16, tag="w1sb")
        w2sb = stat.tile([128, NI, HK], BF16, tag="w2sb")
        xdtp_sb = stat.tile([128, BS // 2, 2 * MC], BF16, tag="xdtp_sb")
        idb = stat.tile([128, 128], BF16, tag="idb")
        idf = stat.tile([128, 128], F32, tag="idf")
        g2stack = stat.tile([128, NI, BS], BF16, tag="g2stack")
        outs_sb = stat.tile([128, 3, BS], F32, tag="outs_sb")
        outT_sb = stat.tile([128, 3, HK], F32, tag="outT_sb")
        ones_sb = stat.tile([1, 512], BF16, tag="ones_sb")
        red_scr = stat.tile([128, D], BF16, tag="red_scr")
        nc.vector.memset(ones_sb[:], 1.0)

        # PE warm-up spin: the HAM clock gate starts at 1.2GHz and needs
        # ~3.4us of sustained activity to reach 2.4GHz. The first ~10us of
        # the kernel is DMA-only (startup loads), so burn it on dummy
        # rank-1 matmuls; block 0 then starts at full clock.
        warm_ps = ps_rp.tile([128, 512], F32, tag="rp")
        for _ in range(32):
            nc.tensor.matmul(warm_ps[:], ones_sb[:, 0:128], ones_sb[:],
                             start=True, stop=True)

        def emit_l2(pblk, px2sb, halves=(0, 1)):
            """Layer-2 grams for block `pblk` (software-pipelined one block
            late, emitted in halves interleaved into the next block's L1
            matmul stream). Batch PAIRS: one [128,128] transpose covers two
            batches' [128,64] slabs (even batch -> partitions 0-63, odd ->
            64-127), and one gram matmul against the block-diagonal xdtP
            emits both grams side by side: out[j, 0:41] = even-batch gram
            (+outs_1 in col 40), out[j, 41:82] = odd batch."""
            for h in halves:
                x2t_ps = ps_tr.tile([128, NPR, 128], BF16, tag="x2t")
                for e2 in range(NPR):
                    b8 = h * HB + 2 * e2
                    nc.tensor.transpose(x2t_ps[:, e2, :],
                                        px2sb[:, b8 * D:(b8 + 2) * D], idb[:])
                x2t = x2tp.tile([128, NPR, 128], BF16, tag="x2t_sb")
                nc.scalar.copy(x2t[:], x2t_ps[:])
                g2ps = ps_sm.tile([128, NPR, 2 * MC], F32, tag="sm")
                for e2 in range(NPR):
                    b2 = (pblk * BB + h * HB) // 2 + e2
                    nc.tensor.matmul(g2ps[:, e2, :], x2t[:, e2, :],
                                     xdtp_sb[:, b2, :], start=True, stop=True)
                b0 = pblk * BB + h * HB
                nc.scalar.copy(
                    g2stack[:, :, b0:b0 + HB:2],
                    g2ps[:, :, 0:NI].rearrange("p e i -> p i e"))
                nc.scalar.copy(
                    g2stack[:, :, b0 + 1:b0 + HB:2],
                    g2ps[:, :, MC:MC + NI].rearrange("p e i -> p i e"))
                nc.scalar.copy(outs_sb[:, 1, b0:b0 + HB:2], g2ps[:, :, NI])
                nc.scalar.copy(outs_sb[:, 1, b0 + 1:b0 + HB:2],
                               g2ps[:, :, MC + NI])

        def load_p0(blk):
            tiles = []
            ch = 0
            for g, gsz in enumerate(GRP0S):
                p_t = sufp.tile([128, gsz, F], BF16, tag="suf")
                eng = nc.sync if g % 2 == 0 else nc.scalar
                eng.dma_start(p_t[:], p0suf_d[blk, :, ch:ch + gsz, :])
                tiles.append((ch, gsz, p_t))
                ch += gsz
            return tiles

        def emit_l0_half(x1ps, x1sb, p0_tiles, s):
            """One 512-column half of layer 0: 7 chunk matmuls + the PSUM->
            SBUF x1 copy. Emitted inside the PREVIOUS block's L1 stream so
            x1sb is fully ready before the block boundary and the next
            block's products start without any DVE idle."""
            for (c0, gsz, p_t) in p0_tiles:
                for e in range(gsz):
                    nc.tensor.matmul(
                        x1ps[:, s * 512:(s + 1) * 512],
                        w0sb[:, c0 + e, :],
                        p_t[:, e, s * 512:(s + 1) * 512],
                        start=(c0 + e == 0), stop=(c0 + e == NCH0 - 1))
            nc.scalar.copy(x1sb[:, s * 512:(s + 1) * 512],
                           x1ps[:, s * 512:(s + 1) * 512])

        def alloc_l0(blk):
            x1ps = ps_acc.tile([128, F], F32, tag="acc")
            x1sb = xkp.tile([128, F], BF16, tag="xk")
            return (x1ps, x1sb, p0_queue.pop(0))

        x2sb_prev = None
        p0_queue = [load_p0(0)]
        l0_next = alloc_l0(0)
        for s in range(NMM):
            emit_l0_half(*l0_next[:2], l0_next[2], s)
        for blk in range(NBLK):
            x1ps, x1sb, _ = l0_next

            if blk == 0:
                # behind block-0's p0 load on the scalar ring: the first
                # products aren't stuck behind this 1.25MB transfer, and it
                # still lands well before L1(0) needs it
                nc.scalar.dma_start(w1sb[:], w1_d[:])
                nc.scalar.dma_start(idb[:], idb_d[:])
            elif blk == 1:
                nc.scalar.dma_start(xdtp_sb[:], xdtp_d[:])
                nc.sync.dma_start(w2sb[:], w2_d[:])
                nc.sync.dma_start(idf[:], idf_d[:])

            # REP groups for layer 1: x0 rows broadcast across partitions.
            # Groups 0-5 and 8 via stride-0 DMA; PE_REP groups via PE
            # ones-matmul (rank-1 broadcast) + ACT PSUM copies; POOL_BC
            # groups via the pool's partition_broadcast (started at block
            # top, consumed ~24us later -- hides pool jitter).
            pe_set = set(PE_REP)
            pool_set = set(POOL_BC)

            def gen_pe_rep(g, rg, x0pe):
                for e in range(GRP):
                    for s in range(NMM):
                        st = ps_rp.tile([128, 512], F32, tag="rp")
                        nc.tensor.matmul(
                            st[:], ones_sb[:, 0:128],
                            x0pe[0:1, e, s * 512:(s + 1) * 512],
                            start=True, stop=True)
                        nc.scalar.copy(
                            rg[:, e, s * 512:(s + 1) * 512], st[:])

            # x0pe loads first: tiny, and the pool broadcast + ones-matmuls
            # need them before the rep DMAs queue up ~7MB on the rings
            x0pe_t = {}
            for g in sorted(pe_set | pool_set):
                x0pe = x0pp.tile([1, GRP, F], BF16, tag="x0pe")
                nc.sync.dma_start(
                    x0pe[:], x0f_d[blk:blk + 1, g * GRP:(g + 1) * GRP, :])
                x0pe_t[g] = x0pe
            rep_grps = {}
            for g in range(NGRP):
                pool = repbc if g in pool_set else repp
                rg = pool.tile([128, GRP, F], BF16, tag="rep")
                if g not in pe_set and g not in pool_set:
                    # ring balance: sync also carries p0 g0 (1.05MB), so g8
                    # rides the scalar ring despite being even
                    eng = nc.sync if g in (0, 2, 4, 6) else nc.scalar
                    eng.dma_start(
                        rg[:], x0f_d[blk:blk + 1, g * GRP:(g + 1) * GRP, :]
                        .partition_broadcast(128))
                rep_grps[g] = rg
            for g in sorted(pool_set):
                nc.gpsimd.partition_broadcast(rep_grps[g][:], x0pe_t[g][:])
            for g in sorted(pe_set):
                gen_pe_rep(g, rep_grps[g], x0pe_t[g])
            # prefetch p0 TWO blocks ahead behind this block's rep traffic:
            # the ring runs ~a full block deep, so a one-block prefetch
            # arrives exactly at its deadline -- two blocks gives ~10us slack.
            # (block 0 also queues p0(1) here, AFTER its rep DMAs, so the
            # first rep groups aren't delayed behind 1.84MB of prefetch)
            if blk == 0:
                p0_queue.append(load_p0(1))
            if blk + 2 < NBLK:
                p0_queue.append(load_p0(blk + 2))

            # ---- layer 1 ----
            # previous block's layer-2 work is emitted in halves inside the
            # L1 stream (after groups 5 and 8) so its transposes/grams mix
            # into dense matmul traffic instead of piling up at the boundary
            x2ps = ps_acc.tile([128, F], F32, tag="acc")
            for g in range(NGRP):
                if x2sb_prev is not None and g in (5, 8):
                    emit_l2(blk - 1, x2sb_prev[:], halves=(0 if g == 5 else 1,))
                # next block's layer 0 rides inside this block's L1 stream
                # (PE has ~0.5us idle per DVE-paced product group): x1sb of
                # block blk+1 is complete before the boundary
                if blk + 1 < NBLK and g in (6, 7):
                    if g == 6:
                        l0_next = alloc_l0(blk + 1)
                    emit_l0_half(*l0_next[:2], l0_next[2], g - 6)
                p_t = pp.tile([128, GRP, F], BF16, tag="p")
                # products split per 512-half so each half's matmuls start
                # as soon as that half's product lands (pp bufs=2 keeps
                # two groups in flight)
                for s in range(NMM):
                    sl = slice(s * 512, (s + 1) * 512)
                    nc.vector.tensor_mul(
                        p_t[:, :, sl],
                        x1sb[:, sl].unsqueeze(1).broadcast_to([128, GRP, 512]),
                        rep_grps[g][:, :, sl])
                for e in range(GRP):
                    i = g * GRP + e
                    for s in range(NMM):
                        nc.tensor.matmul(
                            x2ps[:, s * 512:(s + 1) * 512],
                            w1sb[:, i, :],
                            p_t[:, e, s * 512:(s + 1) * 512],
                            start=(i == 0), stop=(i == NI - 1))
            # outs_0 via ACT Copy+accum_out (one per batch): keeps the sum
            # off the DVE, which paces the whole block
            for b in range(BB):
                nc.scalar.activation(
                    red_scr[:], x1sb[:, b * D:(b + 1) * D],
                    mybir.ActivationFunctionType.Copy,
                    accum_out=outs_sb[:, 0, blk * BB + b:blk * BB + b + 1])
            x2sb = xkp.tile([128, F], BF16, tag="xk")
            nc.scalar.copy(x2sb[:], x2ps[:])
            x2sb_prev = x2sb

        emit_l2(NBLK - 1, x2sb_prev[:])

        # ---- outs_2 = W2 : G2 ----
        out2ps = ps_sm.tile([HK, BS], F32, tag="sm")
        for i in range(NI):
            nc.tensor.matmul(out2ps[:], w2sb[:, i, :], g2stack[:, i, :],
                             start=(i == 0), stop=(i == NI - 1))
        nc.scalar.copy(outs_sb[:, 2, :], out2ps[:])

        # ---- transpose [h, b] -> [b, h] and store ----
        for k in range(3):
            trp = ps_sm.tile([128, 128], F32, tag="sm")
            nc.tensor.transpose(trp[:], outs_sb[:, k, :], idf[:])
            nc.scalar.copy(outT_sb[:, k, :], trp[:])
        nc.sync.dma_start(out_d[:], outT_sb[:])

    nc.compile()
    return nc


_II0, _JJ0 = np.triu_indices(M)          # 820 pairs, i <= j


def _host_prep(x, W0, W1, W2):
    """Build per-core input maps. All reshapes/casts in numpy."""
    # layer-0 symmetric weights: W0s[c,h] = W0[i,j,h] + W0[j,i,h] (i<j), diag 1x
    w0sym = W0[_II0, _JJ0, :] + np.where(
        (_II0 != _JJ0)[:, None], W0[_JJ0, _II0, :], 0.0)          # [820, HK]
    w0pad = np.zeros((NCH0 * 128, HK), np.float32)
    w0pad[:NP0] = w0sym
    # partition-major: [128, NCH0, HK] so the load is one fat run per partition
    w0p = np.ascontiguousarray(
        w0pad.reshape(NCH0, 128, HK).transpose(1, 0, 2)).astype(NPBF16)
    w1t = np.ascontiguousarray(W1.transpose(1, 0, 2)).astype(NPBF16)
    w2t = np.ascontiguousarray(W2.transpose(1, 0, 2)).astype(NPBF16)
    idb = np.eye(128, dtype=np.float32).astype(NPBF16)
    idf = np.eye(128, dtype=np.float32)

    # padded pair index maps (pad rows produce zero products)
    ii = np.zeros(NCH0 * 128, np.int64)
    jj = np.zeros(NCH0 * 128, np.int64)
    ii[:NP0] = _II0
    jj[:NP0] = _JJ0
    pad_mask = np.zeros((NCH0 * 128, 1), np.float32)
    pad_mask[:NP0] = 1.0

    xbf = x.astype(NPBF16)
    in_maps = []
    for c in range(NCORE):
        xs = xbf[c * BS:(c + 1) * BS]                     # [BS, M, D]
        xsT = xs.transpose(1, 0, 2)                       # [M, BS, D]
        xf = xsT.reshape(M, NBLK, F).astype(np.float32)   # [M, NBLK, F]
        x0f = np.ascontiguousarray(
            xf.transpose(1, 0, 2)).astype(NPBF16)         # [NBLK, M, F]
        # host-premultiplied pair-product image, partition-major:
        # [NCH0*128 rows, NBLK, F] -> [NBLK, 128, NCH0, F]
        xfb = xf.astype(NPBF16).astype(np.float32)
        p0 = (xfb[jj] * xfb[ii] * pad_mask[:, :, None]).transpose(1, 0, 2)
        p0suf = np.ascontiguousarray(
            p0.reshape(NBLK, NCH0, 128, F).transpose(0, 2, 1, 3)
        ).astype(NPBF16)
        # block-diagonal paired gram operand: [128, BS/2, 2*MC]
        # partitions 0-63 (d) x cols 0-40: even batch (i cols + ones col);
        # partitions 64-127 x cols 41-81: odd batch.
        xdtp = np.zeros((128, BS // 2, 2 * MC), np.float32)
        xdt_full = np.concatenate(
            [xs.transpose(2, 0, 1).astype(np.float32),
             np.ones((D, BS, 1), np.float32)], axis=2)    # [D, BS, MC]
        xdtp[:D, :, :MC] = xdt_full[:, 0::2, :]
        xdtp[D:, :, MC:] = xdt_full[:, 1::2, :]
        xdtp = np.ascontiguousarray(xdtp).astype(NPBF16)
        in_maps.append({
            "p0suf": p0suf, "x0f": x0f, "xdtp": xdtp,
            "w0": w0p, "w1": w1t, "w2": w2t,
            "idb": idb, "idf": idf,
        })
    return in_maps


def kernel(x, W0, W1, W2):
    global _nc_cache, _last_results
    x = np.asarray(x, dtype=np.float32)
    W0 = np.asarray(W0, dtype=np.float32)
    W1 = np.asarray(W1, dtype=np.float32)
    W2 = np.asarray(W2, dtype=np.float32)

    if _nc_cache is None:
        _nc_cache = _build()
    nc = _nc_cache

    in_maps = _host_prep(x, W0, W1, W2)
    res = run_bass_kernel_spmd(nc, in_maps, list(range(NCORE)),
                               trace=_PROFILE, **_TRACE_KW)
    _last_results = res
    out = np.concatenate(
        [np.asarray(res.results[c]["out"]) for c in range(NCORE)], axis=0)
    return out.astype(np.float32)


# revision 28
# speedup vs baseline: 1.1240x; 1.1240x over previous
"""Trainium2 Bass kernel for nn_CIN (Compressed Interaction Network).

Math (per layer k, x0 = x fixed):
    x_{k+1}[b,h,d] = sum_{i,j} W[i,j,h] * x0[b,i,d] * xk[b,j,d]
    outs_k[b,h]    = sum_d x_{k+1}[b,h,d]
    output = concat(outs_0, outs_1, outs_2)   # [B, 384]

Strategy (pure data parallel over batch, 8 cores x 128 batches):
  - bf16 compute, fp32 PSUM accumulation (fp8 was simulated at rel err
    0.033-0.067 -- fails the 2e-2 gate, so bf16 everywhere).
  - Per core, 8 blocks of 16 batches; free dim F = 16*64 = 1024 (b,d).
  - Layer 0 uses the i<=j symmetry: 820 unique pairs, W0sym = W0[i,j]+W0[j,i],
    and the pair PRODUCTS are built on the host (p0suf = x[i]*x[j] gather
    image, bf16, partition-major): halves the suf DMA bytes vs shipping two
    operand images and removes the L0 DVE products entirely.
  - Layer 1 products P[(i,j), f] = x0[i,f]*x1[j,f] on DVE (2x bf16 mode,
    ~2.13us per 4-chunk group). x0 rows broadcast across partitions (REP):
    7 groups via DMA stride-0 partition-broadcast APs alternating the two
    HWDGE rings, 2 groups (6,7) via PE ones-matmul + ACT PSUM copy, and the
    last group (9) via the pool engine's partition_broadcast -- its ~8us
    (up to ~21us jitter) runtime hides behind a ~24us deadline, relieving
    both the DMA rings and the PE. outs_0 reduce also rides on the pool.
  - Matmuls: stationary = W chunk [c,h], moving = P chunk [c, 512] (PSUM
    write cap), emitted 512-half-first in layer 0 so the x1 copy / first
    L1 product chain overlaps the second half's matmuls. PSUM accumulation
    -> x_{k+1} in [h, (b,d)] layout = next layer's input layout.
  - Layer 2 never materializes x3: outs_2 = W2 : G2 where
    G2'[b][j,i] = sum_d x2[b,j,d]*x0[b,i,d] (per-batch Gram via PE), then
    one 40-chunk contraction. Batch PAIRS share one 128-row transpose and
    one gram matmul against a host-built block-diagonal xdtP (even batch in
    partitions 0-63 / cols 0-40, odd in 64-127 / 41-81), halving the L2 PE
    instruction count; a ones-column per half makes the gram emit outs_1
    for free.
  - Engine budget per block (~modeled): PE ~27us (14 L0 + 80 L1 + 16 REP
    matmuls + 8 transposes + 8 grams), DMA ~25.5us (1.84MB p0 + 7x1.05MB
    rep), DVE ~21.3us (10 product groups), pool ~10us, ACT ~14us. A
    32-matmul warm-up spin covers the DMA-only startup window (PE clock
    ramps 1.2->2.4GHz over ~3.4us of sustained activity).
"""
import os
import sys

sys.path.insert(0, "/opt/trn_rl_repo")
os.environ.setdefault("JAX_PLATFORMS", "cpu")

from contextlib import ExitStack

import numpy as np
import ml_dtypes

import concourse.bass as bass  # noqa: F401
import concourse.tile as tile
from concourse import bacc, library_config, mybir
from concourse.bass_utils import run_bass_kernel_spmd

BF16 = mybir.dt.bfloat16
F32 = mybir.dt.float32
NPBF16 = ml_dtypes.bfloat16

B, M, D, HK = 1024, 40, 64, 128
NCORE = 8
BS = B // NCORE          # 128 batches per core
NBLK = 8                 # blocks per core
BB = BS // NBLK          # 16 batches per block
F = BB * D               # 1024 free elements per block
NI = M                   # 40 chunks in layer 1
GRP = 4                  # chunks fused per group (one TT / one REP DMA)
NGRP = NI // GRP         # 10 groups in layer 1
NP0 = M * (M + 1) // 2   # 820 unique layer-0 pairs
NCH0 = 7                 # layer-0 chunks (820 -> 896 rows, 76 zero-pad)
GRP0S = (4, 3)           # layer-0 group sizes (4 + 3 chunks)
NMM = F // 512           # matmuls per chunk (PSUM write = 512 els max)
HB = BB // 2             # L2 gram half-size (8 batches -> 4 pair-grams)
NPR = HB // 2            # batch pairs per L2 half-call
MC = M + 1               # gram columns per batch (40 i's + ones)
PE_REP = (7,)            # REP groups built by PE ones-matmul
POOL_BC = (9,)           # REP groups built by pool partition_broadcast

_PROFILE = False
_TRACE_KW = {}
_nc_cache = None
_last_results = None


def _build():
    nc = bacc.Bacc("TRN2", target_bir_lowering=False, debug=False,
                   enable_asserts=False)

    p0suf_d = nc.dram_tensor("p0suf", [NBLK, 128, NCH0, F], BF16,
                             kind="ExternalInput").ap()
    x0f_d = nc.dram_tensor("x0f", [NBLK, NI, F], BF16, kind="ExternalInput").ap()
    xdtp_d = nc.dram_tensor("xdtp", [128, BS // 2, 2 * MC], BF16,
                            kind="ExternalInput").ap()
    w0_d = nc.dram_tensor("w0", [128, NCH0, HK], BF16, kind="ExternalInput").ap()
    w1_d = nc.dram_tensor("w1", [128, NI, HK], BF16, kind="ExternalInput").ap()
    w2_d = nc.dram_tensor("w2", [128, NI, HK], BF16, kind="ExternalInput").ap()
    idb_d = nc.dram_tensor("idb", [128, 128], BF16, kind="ExternalInput").ap()
    idf_d = nc.dram_tensor("idf", [128, 128], F32, kind="ExternalInput").ap()
    out_d = nc.dram_tensor("out", [BS, 3 * HK], F32, kind="ExternalOutput").ap()

    with tile.TileContext(nc) as tc, ExitStack() as ctx:
        stat = ctx.enter_context(tc.tile_pool(name="stat", bufs=1))
        sufp = ctx.enter_context(tc.tile_pool(name="sufp", bufs=4))
        # one buffer per non-pool rep group: the PE-REP ACT copies and the
        # trailing DMA groups must never wait on a buffer still being
        # consumed by this block's early products (that stall resets the PE
        # clock ramp); pool-bcast rep tiles get their own pool so the
        # broadcast can start at block top.
        repp = ctx.enter_context(tc.tile_pool(name="repp", bufs=8))
        repbc = ctx.enter_context(tc.tile_pool(name="repbc", bufs=2))
        x0pp = ctx.enter_context(tc.tile_pool(name="x0pp", bufs=2))
        pp = ctx.enter_context(tc.tile_pool(name="pp", bufs=2))
        xkp = ctx.enter_context(tc.tile_pool(name="xkp", bufs=4))
        x2tp = ctx.enter_context(tc.tile_pool(name="x2tp", bufs=2))
        ps_acc = ctx.enter_context(tc.tile_pool(name="ps_acc", bufs=2, space="PSUM"))
        ps_tr = ctx.enter_context(tc.tile_pool(name="ps_tr", bufs=1, space="PSUM"))
        ps_sm = ctx.enter_context(tc.tile_pool(name="ps_sm", bufs=1, space="PSUM"))
        ps_rp = ctx.enter_context(tc.tile_pool(name="ps_rp", bufs=2, space="PSUM"))

        # critical loads first (w0/w1 are fat contiguous descriptors);
        # remaining static tensors stream in behind block-0/1 traffic
        w0sb = stat.tile([128, NCH0, HK], BF16, tag="w0sb")
        nc.sync.dma_start(w0sb[:], w0_d[:])
        w1sb = stat.tile([128, NI, HK], BF16, tag="w1sb")
        w2sb = stat.tile([128, NI, HK], BF16, tag="w2sb")
        xdtp_sb = stat.tile([128, BS // 2, 2 * MC], BF16, tag="xdtp_sb")
        idb = stat.tile([128, 128], BF16, tag="idb")
        idf = stat.tile([128, 128], F32, tag="idf")
        g2stack = stat.tile([128, NI, BS], BF16, tag="g2stack")
        outs_sb = stat.tile([128, 3, BS], F32, tag="outs_sb")
        outT_sb = stat.tile([128, 3, HK], F32, tag="outT_sb")
        ones_sb = stat.tile([1, 512], BF16, tag="ones_sb")
        red_scr = stat.tile([128, D], BF16, tag="red_scr")
        nc.vector.memset(ones_sb[:], 1.0)

        # PE warm-up spin: the HAM clock gate starts at 1.2GHz and needs
        # ~3.4us of sustained activity to reach 2.4GHz. The first ~10us of
        # the kernel is DMA-only (startup loads), so burn it on dummy
        # rank-1 matmuls; block 0 then starts at full clock.
        warm_ps = ps_rp.tile([128, 512], F32, tag="rp")
        for _ in range(32):
            nc.tensor.matmul(warm_ps[:], ones_sb[:, 0:128], ones_sb[:],
                             start=True, stop=True)

        def emit_l2(pblk, px2sb, halves=(0, 1)):
            """Layer-2 grams for block `pblk` (software-pipelined one block
            late, emitted in halves interleaved into the next block's L1
            matmul stream). Batch PAIRS: one [128,128] transpose covers two
            batches' [128,64] slabs (even batch -> partitions 0-63, odd ->
            64-127), and one gram matmul against the block-diagonal xdtP
            emits both grams side by side: out[j, 0:41] = even-batch gram
            (+outs_1 in col 40), out[j, 41:82] = odd batch."""
            for h in halves:
                x2t_ps = ps_tr.tile([128, NPR, 128], BF16, tag="x2t")
                for e2 in range(NPR):
                    b8 = h * HB + 2 * e2
                    nc.tensor.transpose(x2t_ps[:, e2, :],
                                        px2sb[:, b8 * D:(b8 + 2) * D], idb[:])
                x2t = x2tp.tile([128, NPR, 128], BF16, tag="x2t_sb")
                nc.scalar.copy(x2t[:], x2t_ps[:])
                g2ps = ps_sm.tile([128, NPR, 2 * MC], F32, tag="sm")
                for e2 in range(NPR):
                    b2 = (pblk * BB + h * HB) // 2 + e2
                    nc.tensor.matmul(g2ps[:, e2, :], x2t[:, e2, :],
                                     xdtp_sb[:, b2, :], start=True, stop=True)
                b0 = pblk * BB + h * HB
                nc.scalar.copy(
                    g2stack[:, :, b0:b0 + HB:2],
                    g2ps[:, :, 0:NI].rearrange("p e i -> p i e"))
                nc.scalar.copy(
                    g2stack[:, :, b0 + 1:b0 + HB:2],
                    g2ps[:, :, MC:MC + NI].rearrange("p e i -> p i e"))
                nc.scalar.copy(outs_sb[:, 1, b0:b0 + HB:2], g2ps[:, :, NI])
                nc.scalar.copy(outs_sb[:, 1, b0 + 1:b0 + HB:2],
                               g2ps[:, :, MC + NI])

        def load_p0(blk):
            tiles = []
            ch = 0
            for g, gsz in enumerate(GRP0S):
                p_t = sufp.tile([128, gsz, F], BF16, tag="suf")
                eng = nc.sync if g % 2 == 0 else nc.scalar
                eng.dma_start(p_t[:], p0suf_d[blk, :, ch:ch + gsz, :])
                tiles.append((ch, gsz, p_t))
                ch += gsz
            return tiles

        def emit_l0_half(x1ps, x1sb, p0_tiles, s):
            """One 512-column half of layer 0: 7 chunk matmuls + the PSUM->
            SBUF x1 copy. Emitted inside the PREVIOUS block's L1 stream so
            x1sb is fully ready before the block boundary and the next
            block's products start without any DVE idle."""
            for (c0, gsz, p_t) in p0_tiles:
                for e in range(gsz):
                    nc.tensor.matmul(
                        x1ps[:, s * 512:(s + 1) * 512],
                        w0sb[:, c0 + e, :],
                        p_t[:, e, s * 512:(s + 1) * 512],
                        start=(c0 + e == 0), stop=(c0 + e == NCH0 - 1))
            nc.scalar.copy(x1sb[:, s * 512:(s + 1) * 512],
                           x1ps[:, s * 512:(s + 1) * 512])

        def alloc_l0(blk):
            x1ps = ps_acc.tile([128, F], F32, tag="acc")
            x1sb = xkp.tile([128, F], BF16, tag="xk")
            return (x1ps, x1sb, p0_queue.pop(0))

        x2sb_prev = None
        p0_queue = [load_p0(0)]
        l0_next = alloc_l0(0)
        for s in range(NMM):
            emit_l0_half(*l0_next[:2], l0_next[2], s)
        for blk in range(NBLK):
            x1ps, x1sb, _ = l0_next

            if blk == 0:
                # behind block-0's p0 load on the scalar ring: the first
                # products aren't stuck behind this 1.25MB transfer, and it
                # still lands well before L1(0) needs it
                nc.scalar.dma_start(w1sb[:], w1_d[:])
                nc.scalar.dma_start(idb[:], idb_d[:])
            elif blk == 1:
                nc.scalar.dma_start(xdtp_sb[:], xdtp_d[:])
                nc.sync.dma_start(w2sb[:], w2_d[:])
                nc.sync.dma_start(idf[:], idf_d[:])

            # REP groups for layer 1: x0 rows broadcast across partitions.
            # Groups 0-5 and 8 via stride-0 DMA; PE_REP groups via PE
            # ones-matmul (rank-1 broadcast) + ACT PSUM copies; POOL_BC
            # groups via the pool's partition_broadcast (started at block
            # top, consumed ~24us later -- hides pool jitter).
            pe_set = set(PE_REP)
            pool_set = set(POOL_BC)

            def gen_pe_rep(g, rg, x0pe):
                for e in range(GRP):
                    for s in range(NMM):
                        st = ps_rp.tile([128, 512], F32, tag="rp")
                        nc.tensor.matmul(
                            st[:], ones_sb[:, 0:128],
                            x0pe[0:1, e, s * 512:(s + 1) * 512],
                            start=True, stop=True)
                        nc.scalar.copy(
                            rg[:, e, s * 512:(s + 1) * 512], st[:])

            # x0pe loads first: tiny, and the pool broadcast + ones-matmuls
            # need them before the rep DMAs queue up ~7MB on the rings
            x0pe_t = {}
            for g in sorted(pe_set | pool_set):
                x0pe = x0pp.tile([1, GRP, F], BF16, tag="x0pe")
                nc.sync.dma_start(
                    x0pe[:], x0f_d[blk:blk + 1, g * GRP:(g + 1) * GRP, :])
                x0pe_t[g] = x0pe
            rep_grps = {}
            for g in range(NGRP):
                pool = repbc if g in pool_set else repp
                rg = pool.tile([128, GRP, F], BF16, tag="rep")
                if g not in pe_set and g not in pool_set:
                    # ring balance: sync also carries p0 g0 (1.05MB), so g8
                    # rides the scalar ring despite being even
                    eng = nc.sync if g in (0, 2, 4, 6) else nc.scalar
                    eng.dma_start(
                        rg[:], x0f_d[blk:blk + 1, g * GRP:(g + 1) * GRP, :]
                        .partition_broadcast(128))
                rep_grps[g] = rg
            for g in sorted(pool_set):
                nc.gpsimd.partition_broadcast(rep_grps[g][:], x0pe_t[g][:])
            for g in sorted(pe_set):
                gen_pe_rep(g, rep_grps[g], x0pe_t[g])
            # prefetch p0 TWO blocks ahead behind this block's rep traffic:
            # the ring runs ~a full block deep, so a one-block prefetch
            # arrives exactly at its deadline -- two blocks gives ~10us slack.
            # (block 0 also queues p0(1) here, AFTER its rep DMAs, so the
            # first rep groups aren't delayed behind 1.84MB of prefetch)
            if blk == 0:
                p0_queue.append(load_p0(1))
            if blk + 2 < NBLK:
                p0_queue.append(load_p0(blk + 2))

            # ---- layer 1 ----
            # previous block's layer-2 work is emitted in halves inside the
            # L1 stream (after groups 5 and 8) so its transposes/grams mix
            # into dense matmul traffic instead of piling up at the boundary
            x2ps = ps_acc.tile([128, F], F32, tag="acc")
            for g in range(NGRP):
                if x2sb_prev is not None and g in (5, 8):
                    emit_l2(blk - 1, x2sb_prev[:], halves=(0 if g == 5 else 1,))
                # next block's layer 0 rides inside this block's L1 stream
                # (PE has ~0.5us idle per DVE-paced product group): x1sb of
                # block blk+1 is complete before the boundary
                if blk + 1 < NBLK and g in (6, 7):
                    if g == 6:
                        l0_next = alloc_l0(blk + 1)
                    emit_l0_half(*l0_next[:2], l0_next[2], g - 6)
                p_t = pp.tile([128, GRP, F], BF16, tag="p")
                # products split per 512-half so each half's matmuls start
                # as soon as that half's product lands (pp bufs=2 keeps
                # two groups in flight)
                for s in range(NMM):
                    sl = slice(s * 512, (s + 1) * 512)
                    nc.vector.tensor_mul(
                        p_t[:, :, sl],
                        x1sb[:, sl].unsqueeze(1).broadcast_to([128, GRP, 512]),
                        rep_grps[g][:, :, sl])
                for e in range(GRP):
                    i = g * GRP + e
                    for s in range(NMM):
                        nc.tensor.matmul(
                            x2ps[:, s * 512:(s + 1) * 512],
                            w1sb[:, i, :],
                            p_t[:, e, s * 512:(s + 1) * 512],
                            start=(i == 0), stop=(i == NI - 1))
            # outs_0 reduce emitted late so the DVE queue serves products
            # first (an ACT accum_out variant delays x2copy and cascades)
            nc.vector.tensor_reduce(
                outs_sb[:, 0, blk * BB:(blk + 1) * BB],
                x1sb[:].rearrange("p (b d) -> p b d", d=D),
                axis=mybir.AxisListType.X, op=mybir.AluOpType.add)
            x2sb = xkp.tile([128, F], BF16, tag="xk")
            nc.scalar.copy(x2sb[:], x2ps[:])
            x2sb_prev = x2sb

        emit_l2(NBLK - 1, x2sb_prev[:])

        # ---- outs_2 = W2 : G2 ----
        out2ps = ps_sm.tile([HK, BS], F32, tag="sm")
        for i in range(NI):
            nc.tensor.matmul(out2ps[:], w2sb[:, i, :], g2stack[:, i, :],
                             start=(i == 0), stop=(i == NI - 1))
        nc.scalar.copy(outs_sb[:, 2, :], out2ps[:])

        # ---- transpose [h, b] -> [b, h] and store ----
        for k in range(3):
            trp = ps_sm.tile([128, 128], F32, tag="sm")
            nc.tensor.transpose(trp[:], outs_sb[:, k, :], idf[:])
            nc.scalar.copy(outT_sb[:, k, :], trp[:])
        nc.sync.dma_start(out_d[:], outT_sb[:])

    nc.compile()
    return nc


_II0, _JJ0 = np.triu_indices(M)          # 820 pairs, i <= j


def _host_prep(x, W0, W1, W2):
    """Build per-core input maps. All reshapes/casts in numpy."""
    # layer-0 symmetric weights: W0s[c,h] = W0[i,j,h] + W0[j,i,h] (i<j), diag 1x
    w0sym = W0[_II0, _JJ0, :] + np.where(
        (_II0 != _JJ0)[:, None], W0[_JJ0, _II0, :], 0.0)          # [820, HK]
    w0pad = np.zeros((NCH0 * 128, HK), np.float32)
    w0pad[:NP0] = w0sym
    # partition-major: [128, NCH0, HK] so the load is one fat run per partition
    w0p = np.ascontiguousarray(
        w0pad.reshape(NCH0, 128, HK).transpose(1, 0, 2)).astype(NPBF16)
    w1t = np.ascontiguousarray(W1.transpose(1, 0, 2)).astype(NPBF16)
    w2t = np.ascontiguousarray(W2.transpose(1, 0, 2)).astype(NPBF16)
    idb = np.eye(128, dtype=np.float32).astype(NPBF16)
    idf = np.eye(128, dtype=np.float32)

    # padded pair index maps (pad rows produce zero products)
    ii = np.zeros(NCH0 * 128, np.int64)
    jj = np.zeros(NCH0 * 128, np.int64)
    ii[:NP0] = _II0
    jj[:NP0] = _JJ0
    pad_mask = np.zeros((NCH0 * 128, 1), np.float32)
    pad_mask[:NP0] = 1.0

    xbf = x.astype(NPBF16)
    in_maps = []
    for c in range(NCORE):
        xs = xbf[c * BS:(c + 1) * BS]                     # [BS, M, D]
        xsT = xs.transpose(1, 0, 2)                       # [M, BS, D]
        xf = xsT.reshape(M, NBLK, F).astype(np.float32)   # [M, NBLK, F]
        x0f = np.ascontiguousarray(
            xf.transpose(1, 0, 2)).astype(NPBF16)         # [NBLK, M, F]
        # host-premultiplied pair-product image, partition-major:
        # [NCH0*128 rows, NBLK, F] -> [NBLK, 128, NCH0, F]
        xfb = xf.astype(NPBF16).astype(np.float32)
        p0 = (xfb[jj] * xfb[ii] * pad_mask[:, :, None]).transpose(1, 0, 2)
        p0suf = np.ascontiguousarray(
            p0.reshape(NBLK, NCH0, 128, F).transpose(0, 2, 1, 3)
        ).astype(NPBF16)
        # block-diagonal paired gram operand: [128, BS/2, 2*MC]
        # partitions 0-63 (d) x cols 0-40: even batch (i cols + ones col);
        # partitions 64-127 x cols 41-81: odd batch.
        xdtp = np.zeros((128, BS // 2, 2 * MC), np.float32)
        xdt_full = np.concatenate(
            [xs.transpose(2, 0, 1).astype(np.float32),
             np.ones((D, BS, 1), np.float32)], axis=2)    # [D, BS, MC]
        xdtp[:D, :, :MC] = xdt_full[:, 0::2, :]
        xdtp[D:, :, MC:] = xdt_full[:, 1::2, :]
        xdtp = np.ascontiguousarray(xdtp).astype(NPBF16)
        in_maps.append({
            "p0suf": p0suf, "x0f": x0f, "xdtp": xdtp,
            "w0": w0p, "w1": w1t, "w2": w2t,
            "idb": idb, "idf": idf,
        })
    return in_maps


def kernel(x, W0, W1, W2):
    global _nc_cache, _last_results
    x = np.asarray(x, dtype=np.float32)
    W0 = np.asarray(W0, dtype=np.float32)
    W1 = np.asarray(W1, dtype=np.float32)
    W2 = np.asarray(W2, dtype=np.float32)

    if _nc_cache is None:
        _nc_cache = _build()
    nc = _nc_cache

    in_maps = _host_prep(x, W0, W1, W2)
    res = run_bass_kernel_spmd(nc, in_maps, list(range(NCORE)),
                               trace=_PROFILE, **_TRACE_KW)
    _last_results = res
    out = np.concatenate(
        [np.asarray(res.results[c]["out"]) for c in range(NCORE)], axis=0)
    return out.astype(np.float32)


# revision 29
# speedup vs baseline: 1.1437x; 1.0175x over previous
"""Trainium2 Bass kernel for nn_CIN (Compressed Interaction Network).

Math (per layer k, x0 = x fixed):
    x_{k+1}[b,h,d] = sum_{i,j} W[i,j,h] * x0[b,i,d] * xk[b,j,d]
    outs_k[b,h]    = sum_d x_{k+1}[b,h,d]
    output = concat(outs_0, outs_1, outs_2)   # [B, 384]

Strategy (pure data parallel over batch, 8 cores x 128 batches):
  - bf16 compute, fp32 PSUM accumulation (fp8 was simulated at rel err
    0.033-0.067 -- fails the 2e-2 gate, so bf16 everywhere).
  - Per core, 8 blocks of 16 batches; free dim F = 16*64 = 1024 (b,d).
  - Layer 0 uses the i<=j symmetry: 820 unique pairs, W0sym = W0[i,j]+W0[j,i],
    and the pair PRODUCTS are built on the host (p0suf = x[i]*x[j] gather
    image, bf16, partition-major): halves the suf DMA bytes vs shipping two
    operand images and removes the L0 DVE products entirely.
  - Layer 1 products P[(i,j), f] = x0[i,f]*x1[j,f] on DVE (2x bf16 mode,
    ~2.13us per 4-chunk group). x0 rows broadcast across partitions (REP):
    7 groups via DMA stride-0 partition-broadcast APs alternating the two
    HWDGE rings, 2 groups (6,7) via PE ones-matmul + ACT PSUM copy, and the
    last group (9) via the pool engine's partition_broadcast -- its ~8us
    (up to ~21us jitter) runtime hides behind a ~24us deadline, relieving
    both the DMA rings and the PE. outs_0 reduce also rides on the pool.
  - Matmuls: stationary = W chunk [c,h], moving = P chunk [c, 512] (PSUM
    write cap), emitted 512-half-first in layer 0 so the x1 copy / first
    L1 product chain overlaps the second half's matmuls. PSUM accumulation
    -> x_{k+1} in [h, (b,d)] layout = next layer's input layout.
  - Layer 2 never materializes x3: outs_2 = W2 : G2 where
    G2'[b][j,i] = sum_d x2[b,j,d]*x0[b,i,d] (per-batch Gram via PE), then
    one 40-chunk contraction. Batch PAIRS share one 128-row transpose and
    one gram matmul against a host-built block-diagonal xdtP (even batch in
    partitions 0-63 / cols 0-40, odd in 64-127 / 41-81), halving the L2 PE
    instruction count; a ones-column per half makes the gram emit outs_1
    for free.
  - Engine budget per block (~modeled): PE ~27us (14 L0 + 80 L1 + 16 REP
    matmuls + 8 transposes + 8 grams), DMA ~25.5us (1.84MB p0 + 7x1.05MB
    rep), DVE ~21.3us (10 product groups), pool ~10us, ACT ~14us. A
    32-matmul warm-up spin covers the DMA-only startup window (PE clock
    ramps 1.2->2.4GHz over ~3.4us of sustained activity).
"""
import os
import sys

sys.path.insert(0, "/opt/trn_rl_repo")
os.environ.setdefault("JAX_PLATFORMS", "cpu")

from contextlib import ExitStack

import numpy as np
import ml_dtypes

import concourse.bass as bass  # noqa: F401
import concourse.tile as tile
from concourse import bacc, library_config, mybir
from concourse.bass_utils import run_bass_kernel_spmd

BF16 = mybir.dt.bfloat16
F32 = mybir.dt.float32
NPBF16 = ml_dtypes.bfloat16

B, M, D, HK = 1024, 40, 64, 128
NCORE = 8
BS = B // NCORE          # 128 batches per core
NBLK = 8                 # blocks per core
BB = BS // NBLK          # 16 batches per block
F = BB * D               # 1024 free elements per block
NI = M                   # 40 chunks in layer 1
GRP = 4                  # chunks fused per group (one TT / one REP DMA)
NGRP = NI // GRP         # 10 groups in layer 1
NP0 = M * (M + 1) // 2   # 820 unique layer-0 pairs
NCH0 = 7                 # layer-0 chunks (820 -> 896 rows, 76 zero-pad)
GRP0S = (4, 3)           # layer-0 group sizes (4 + 3 chunks)
NMM = F // 512           # matmuls per chunk (PSUM write = 512 els max)
HB = BB // 2             # L2 gram half-size (8 batches -> 4 pair-grams)
NPR = HB // 2            # batch pairs per L2 half-call
MC = M + 1               # gram columns per batch (40 i's + ones)
PE_REP = (7,)            # REP groups built by PE ones-matmul
POOL_BC = (9,)           # REP groups built by pool partition_broadcast

_PROFILE = False
_TRACE_KW = {}
_nc_cache = None
_last_results = None


def _build():
    nc = bacc.Bacc("TRN2", target_bir_lowering=False, debug=False,
                   enable_asserts=False)

    p0suf_d = nc.dram_tensor("p0suf", [NBLK, 128, NCH0, F], BF16,
                             kind="ExternalInput").ap()
    x0f_d = nc.dram_tensor("x0f", [NBLK, NI, F], BF16, kind="ExternalInput").ap()
    xdtp_d = nc.dram_tensor("xdtp", [128, BS // 2, 2 * MC], BF16,
                            kind="ExternalInput").ap()
    w0_d = nc.dram_tensor("w0", [128, NCH0, HK], BF16, kind="ExternalInput").ap()
    w1_d = nc.dram_tensor("w1", [128, NI, HK], BF16, kind="ExternalInput").ap()
    w2_d = nc.dram_tensor("w2", [128, NI, HK], BF16, kind="ExternalInput").ap()
    idb_d = nc.dram_tensor("idb", [128, 128], BF16, kind="ExternalInput").ap()
    idf_d = nc.dram_tensor("idf", [128, 128], F32, kind="ExternalInput").ap()
    out_d = nc.dram_tensor("out", [BS, 3 * HK], F32, kind="ExternalOutput").ap()

    with tile.TileContext(nc) as tc, ExitStack() as ctx:
        stat = ctx.enter_context(tc.tile_pool(name="stat", bufs=1))
        sufp = ctx.enter_context(tc.tile_pool(name="sufp", bufs=4))
        # one buffer per non-pool rep group: the PE-REP ACT copies and the
        # trailing DMA groups must never wait on a buffer still being
        # consumed by this block's early products (that stall resets the PE
        # clock ramp); pool-bcast rep tiles get their own pool so the
        # broadcast can start at block top.
        repp = ctx.enter_context(tc.tile_pool(name="repp", bufs=8))
        repbc = ctx.enter_context(tc.tile_pool(name="repbc", bufs=2))
        x0pp = ctx.enter_context(tc.tile_pool(name="x0pp", bufs=2))
        pp = ctx.enter_context(tc.tile_pool(name="pp", bufs=2))
        xkp = ctx.enter_context(tc.tile_pool(name="xkp", bufs=4))
        x2tp = ctx.enter_context(tc.tile_pool(name="x2tp", bufs=2))
        ps_acc = ctx.enter_context(tc.tile_pool(name="ps_acc", bufs=2, space="PSUM"))
        ps_tr = ctx.enter_context(tc.tile_pool(name="ps_tr", bufs=1, space="PSUM"))
        ps_sm = ctx.enter_context(tc.tile_pool(name="ps_sm", bufs=1, space="PSUM"))
        ps_rp = ctx.enter_context(tc.tile_pool(name="ps_rp", bufs=2, space="PSUM"))

        # critical loads first (w0/w1 are fat contiguous descriptors);
        # remaining static tensors stream in behind block-0/1 traffic
        w0sb = stat.tile([128, NCH0, HK], BF16, tag="w0sb")
        nc.sync.dma_start(w0sb[:], w0_d[:])
        w1sb = stat.tile([128, NI, HK], BF16, tag="w1sb")
        w2sb = stat.tile([128, NI, HK], BF16, tag="w2sb")
        xdtp_sb = stat.tile([128, BS // 2, 2 * MC], BF16, tag="xdtp_sb")
        idb = stat.tile([128, 128], BF16, tag="idb")
        idf = stat.tile([128, 128], F32, tag="idf")
        g2stack = stat.tile([128, NI, BS], BF16, tag="g2stack")
        outs_sb = stat.tile([128, 3, BS], F32, tag="outs_sb")
        outT_sb = stat.tile([128, 3, HK], F32, tag="outT_sb")
        ones_sb = stat.tile([1, 512], BF16, tag="ones_sb")
        red_scr = stat.tile([128, D], BF16, tag="red_scr")
        nc.vector.memset(ones_sb[:], 1.0)

        # PE warm-up spin: the HAM clock gate starts at 1.2GHz and needs
        # ~3.4us of sustained activity to reach 2.4GHz. The first ~10us of
        # the kernel is DMA-only (startup loads), so burn it on dummy
        # rank-1 matmuls; block 0 then starts at full clock.
        warm_ps = ps_rp.tile([128, 512], F32, tag="rp")
        for _ in range(32):
            nc.tensor.matmul(warm_ps[:], ones_sb[:, 0:128], ones_sb[:],
                             start=True, stop=True)

        def emit_l2(pblk, px2sb, halves=(0, 1)):
            """Layer-2 grams for block `pblk` (software-pipelined one block
            late, emitted in halves interleaved into the next block's L1
            matmul stream). Batch PAIRS: one [128,128] transpose covers two
            batches' [128,64] slabs (even batch -> partitions 0-63, odd ->
            64-127), and one gram matmul against the block-diagonal xdtP
            emits both grams side by side: out[j, 0:41] = even-batch gram
            (+outs_1 in col 40), out[j, 41:82] = odd batch."""
            for h in halves:
                x2t_ps = ps_tr.tile([128, NPR, 128], BF16, tag="x2t")
                for e2 in range(NPR):
                    b8 = h * HB + 2 * e2
                    nc.tensor.transpose(x2t_ps[:, e2, :],
                                        px2sb[:, b8 * D:(b8 + 2) * D], idb[:])
                x2t = x2tp.tile([128, NPR, 128], BF16, tag="x2t_sb")
                nc.scalar.copy(x2t[:], x2t_ps[:])
                g2ps = ps_sm.tile([128, NPR, 2 * MC], F32, tag="sm")
                for e2 in range(NPR):
                    b2 = (pblk * BB + h * HB) // 2 + e2
                    nc.tensor.matmul(g2ps[:, e2, :], x2t[:, e2, :],
                                     xdtp_sb[:, b2, :], start=True, stop=True)
                b0 = pblk * BB + h * HB
                nc.scalar.copy(
                    g2stack[:, :, b0:b0 + HB:2],
                    g2ps[:, :, 0:NI].rearrange("p e i -> p i e"))
                nc.scalar.copy(
                    g2stack[:, :, b0 + 1:b0 + HB:2],
                    g2ps[:, :, MC:MC + NI].rearrange("p e i -> p i e"))
                nc.scalar.copy(outs_sb[:, 1, b0:b0 + HB:2], g2ps[:, :, NI])
                nc.scalar.copy(outs_sb[:, 1, b0 + 1:b0 + HB:2],
                               g2ps[:, :, MC + NI])

        def load_p0(blk):
            tiles = []
            ch = 0
            for g, gsz in enumerate(GRP0S):
                p_t = sufp.tile([128, gsz, F], BF16, tag="suf")
                eng = nc.sync if g % 2 == 0 else nc.scalar
                eng.dma_start(p_t[:], p0suf_d[blk, :, ch:ch + gsz, :])
                tiles.append((ch, gsz, p_t))
                ch += gsz
            return tiles

        def emit_l0_half(x1ps, x1sb, p0_tiles, s):
            """One 512-column half of layer 0: 7 chunk matmuls + the PSUM->
            SBUF x1 copy. Emitted inside the PREVIOUS block's L1 stream so
            x1sb is fully ready before the block boundary and the next
            block's products start without any DVE idle."""
            for (c0, gsz, p_t) in p0_tiles:
                for e in range(gsz):
                    nc.tensor.matmul(
                        x1ps[:, s * 512:(s + 1) * 512],
                        w0sb[:, c0 + e, :],
                        p_t[:, e, s * 512:(s + 1) * 512],
                        start=(c0 + e == 0), stop=(c0 + e == NCH0 - 1))
            nc.scalar.copy(x1sb[:, s * 512:(s + 1) * 512],
                           x1ps[:, s * 512:(s + 1) * 512])

        def alloc_l0(blk):
            x1ps = ps_acc.tile([128, F], F32, tag="acc")
            x1sb = xkp.tile([128, F], BF16, tag="xk")
            return (x1ps, x1sb, p0_queue.pop(0))

        x2sb_prev = None
        p0_queue = [load_p0(0), load_p0(1)]
        l0_next = alloc_l0(0)
        for s in range(NMM):
            emit_l0_half(*l0_next[:2], l0_next[2], s)
        for blk in range(NBLK):
            x1ps, x1sb, _ = l0_next

            if blk == 0:
                # behind block-0's p0 load on the scalar ring: the first
                # products aren't stuck behind this 1.25MB transfer, and it
                # still lands well before L1(0) needs it
                nc.scalar.dma_start(w1sb[:], w1_d[:])
                nc.scalar.dma_start(idb[:], idb_d[:])
            elif blk == 1:
                nc.scalar.dma_start(xdtp_sb[:], xdtp_d[:])
                nc.sync.dma_start(w2sb[:], w2_d[:])
                nc.sync.dma_start(idf[:], idf_d[:])

            # REP groups for layer 1: x0 rows broadcast across partitions.
            # Groups 0-5 and 8 via stride-0 DMA; PE_REP groups via PE
            # ones-matmul (rank-1 broadcast) + ACT PSUM copies; POOL_BC
            # groups via the pool's partition_broadcast (started at block
            # top, consumed ~24us later -- hides pool jitter).
            pe_set = set(PE_REP)
            pool_set = set(POOL_BC)

            def gen_pe_rep(g, rg, x0pe):
                for e in range(GRP):
                    for s in range(NMM):
                        st = ps_rp.tile([128, 512], F32, tag="rp")
                        nc.tensor.matmul(
                            st[:], ones_sb[:, 0:128],
                            x0pe[0:1, e, s * 512:(s + 1) * 512],
                            start=True, stop=True)
                        nc.scalar.copy(
                            rg[:, e, s * 512:(s + 1) * 512], st[:])

            # x0pe loads first: tiny, and the pool broadcast + ones-matmuls
            # need them before the rep DMAs queue up ~7MB on the rings
            x0pe_t = {}
            for g in sorted(pe_set | pool_set):
                x0pe = x0pp.tile([1, GRP, F], BF16, tag="x0pe")
                nc.sync.dma_start(
                    x0pe[:], x0f_d[blk:blk + 1, g * GRP:(g + 1) * GRP, :])
                x0pe_t[g] = x0pe
            rep_grps = {}
            for g in range(NGRP):
                pool = repbc if g in pool_set else repp
                rg = pool.tile([128, GRP, F], BF16, tag="rep")
                if g not in pe_set and g not in pool_set:
                    # ring balance: sync also carries p0 g0 (1.05MB), so g8
                    # rides the scalar ring despite being even
                    eng = nc.sync if g in (0, 2, 4, 6) else nc.scalar
                    eng.dma_start(
                        rg[:], x0f_d[blk:blk + 1, g * GRP:(g + 1) * GRP, :]
                        .partition_broadcast(128))
                rep_grps[g] = rg
            for g in sorted(pool_set):
                nc.gpsimd.partition_broadcast(rep_grps[g][:], x0pe_t[g][:])
            for g in sorted(pe_set):
                gen_pe_rep(g, rep_grps[g], x0pe_t[g])
            # prefetch p0 TWO blocks ahead behind this block's rep traffic:
            # the ring runs ~a full block deep, so a one-block prefetch
            # arrives exactly at its deadline -- two blocks gives ~10us slack
            if blk + 2 < NBLK:
                p0_queue.append(load_p0(blk + 2))

            # ---- layer 1 ----
            # previous block's layer-2 work is emitted in halves inside the
            # L1 stream (after groups 5 and 8) so its transposes/grams mix
            # into dense matmul traffic instead of piling up at the boundary
            x2ps = ps_acc.tile([128, F], F32, tag="acc")
            for g in range(NGRP):
                if x2sb_prev is not None and g in (5, 8):
                    emit_l2(blk - 1, x2sb_prev[:], halves=(0 if g == 5 else 1,))
                # next block's layer 0 rides inside this block's L1 stream
                # (PE has ~0.5us idle per DVE-paced product group): x1sb of
                # block blk+1 is complete before the boundary
                if blk + 1 < NBLK and g in (6, 7):
                    if g == 6:
                        l0_next = alloc_l0(blk + 1)
                    emit_l0_half(*l0_next[:2], l0_next[2], g - 6)
                p_t = pp.tile([128, GRP, F], BF16, tag="p")
                # products split per 512-half so each half's matmuls start
                # as soon as that half's product lands (pp bufs=2 keeps
                # two groups in flight)
                for s in range(NMM):
                    sl = slice(s * 512, (s + 1) * 512)
                    nc.vector.tensor_mul(
                        p_t[:, :, sl],
                        x1sb[:, sl].unsqueeze(1).broadcast_to([128, GRP, 512]),
                        rep_grps[g][:, :, sl])
                for e in range(GRP):
                    i = g * GRP + e
                    for s in range(NMM):
                        nc.tensor.matmul(
                            x2ps[:, s * 512:(s + 1) * 512],
                            w1sb[:, i, :],
                            p_t[:, e, s * 512:(s + 1) * 512],
                            start=(i == 0), stop=(i == NI - 1))
            # outs_0 reduce emitted late so the DVE queue serves products
            # first (an ACT accum_out variant delays x2copy and cascades)
            nc.vector.tensor_reduce(
                outs_sb[:, 0, blk * BB:(blk + 1) * BB],
                x1sb[:].rearrange("p (b d) -> p b d", d=D),
                axis=mybir.AxisListType.X, op=mybir.AluOpType.add)
            x2sb = xkp.tile([128, F], BF16, tag="xk")
            nc.scalar.copy(x2sb[:], x2ps[:])
            x2sb_prev = x2sb

        emit_l2(NBLK - 1, x2sb_prev[:])

        # ---- outs_2 = W2 : G2 ----
        out2ps = ps_sm.tile([HK, BS], F32, tag="sm")
        for i in range(NI):
            nc.tensor.matmul(out2ps[:], w2sb[:, i, :], g2stack[:, i, :],
                             start=(i == 0), stop=(i == NI - 1))
        nc.scalar.copy(outs_sb[:, 2, :], out2ps[:])

        # ---- transpose [h, b] -> [b, h] and store ----
        for k in range(3):
            trp = ps_sm.tile([128, 128], F32, tag="sm")
            nc.tensor.transpose(trp[:], outs_sb[:, k, :], idf[:])
            nc.scalar.copy(outT_sb[:, k, :], trp[:])
        nc.sync.dma_start(out_d[:], outT_sb[:])

    nc.compile()
    return nc


_II0, _JJ0 = np.triu_indices(M)          # 820 pairs, i <= j


def _host_prep(x, W0, W1, W2):
    """Build per-core input maps. All reshapes/casts in numpy."""
    # layer-0 symmetric weights: W0s[c,h] = W0[i,j,h] + W0[j,i,h] (i<j), diag 1x
    w0sym = W0[_II0, _JJ0, :] + np.where(
        (_II0 != _JJ0)[:, None], W0[_JJ0, _II0, :], 0.0)          # [820, HK]
    w0pad = np.zeros((NCH0 * 128, HK), np.float32)
    w0pad[:NP0] = w0sym
    # partition-major: [128, NCH0, HK] so the load is one fat run per partition
    w0p = np.ascontiguousarray(
        w0pad.reshape(NCH0, 128, HK).transpose(1, 0, 2)).astype(NPBF16)
    w1t = np.ascontiguousarray(W1.transpose(1, 0, 2)).astype(NPBF16)
    w2t = np.ascontiguousarray(W2.transpose(1, 0, 2)).astype(NPBF16)
    idb = np.eye(128, dtype=np.float32).astype(NPBF16)
    idf = np.eye(128, dtype=np.float32)

    # padded pair index maps (pad rows produce zero products)
    ii = np.zeros(NCH0 * 128, np.int64)
    jj = np.zeros(NCH0 * 128, np.int64)
    ii[:NP0] = _II0
    jj[:NP0] = _JJ0
    pad_mask = np.zeros((NCH0 * 128, 1), np.float32)
    pad_mask[:NP0] = 1.0

    xbf = x.astype(NPBF16)
    in_maps = []
    for c in range(NCORE):
        xs = xbf[c * BS:(c + 1) * BS]                     # [BS, M, D]
        xsT = xs.transpose(1, 0, 2)                       # [M, BS, D]
        xf = xsT.reshape(M, NBLK, F).astype(np.float32)   # [M, NBLK, F]
        x0f = np.ascontiguousarray(
            xf.transpose(1, 0, 2)).astype(NPBF16)         # [NBLK, M, F]
        # host-premultiplied pair-product image, partition-major:
        # [NCH0*128 rows, NBLK, F] -> [NBLK, 128, NCH0, F]
        xfb = xf.astype(NPBF16).astype(np.float32)
        p0 = (xfb[jj] * xfb[ii] * pad_mask[:, :, None]).transpose(1, 0, 2)
        p0suf = np.ascontiguousarray(
            p0.reshape(NBLK, NCH0, 128, F).transpose(0, 2, 1, 3)
        ).astype(NPBF16)
        # block-diagonal paired gram operand: [128, BS/2, 2*MC]
        # partitions 0-63 (d) x cols 0-40: even batch (i cols + ones col);
        # partitions 64-127 x cols 41-81: odd batch.
        xdtp = np.zeros((128, BS // 2, 2 * MC), np.float32)
        xdt_full = np.concatenate(
            [xs.transpose(2, 0, 1).astype(np.float32),
             np.ones((D, BS, 1), np.float32)], axis=2)    # [D, BS, MC]
        xdtp[:D, :, :MC] = xdt_full[:, 0::2, :]
        xdtp[D:, :, MC:] = xdt_full[:, 1::2, :]
        xdtp = np.ascontiguousarray(xdtp).astype(NPBF16)
        in_maps.append({
            "p0suf": p0suf, "x0f": x0f, "xdtp": xdtp,
            "w0": w0p, "w1": w1t, "w2": w2t,
            "idb": idb, "idf": idf,
        })
    return in_maps


def kernel(x, W0, W1, W2):
    global _nc_cache, _last_results
    x = np.asarray(x, dtype=np.float32)
    W0 = np.asarray(W0, dtype=np.float32)
    W1 = np.asarray(W1, dtype=np.float32)
    W2 = np.asarray(W2, dtype=np.float32)

    if _nc_cache is None:
        _nc_cache = _build()
    nc = _nc_cache

    in_maps = _host_prep(x, W0, W1, W2)
    res = run_bass_kernel_spmd(nc, in_maps, list(range(NCORE)),
                               trace=_PROFILE, **_TRACE_KW)
    _last_results = res
    out = np.concatenate(
        [np.asarray(res.results[c]["out"]) for c in range(NCORE)], axis=0)
    return out.astype(np.float32)
